# revision 22
# baseline (speedup 1.0000x reference)
"""EquivariantAttention Trainium2 kernel.

B=2, L=2048, D=512, H=8, HD=64 over 8 NeuronCores.
Each core owns ONE batch and TWO heads (core c: batch c//4, heads
{2*(c%4), 2*(c%4)+1}); it computes those heads' attention plus their
partial contribution to the output projection y_c = sum_h Wo_h^T usc_h
over the full sequence.  The host gather sums the 4 per-batch partials
(no on-device collective at all).

Math notes:
  Qi . Ki = ||Q_l||*||K_m|| + Q_l^T C K_m,  C = basis_q[:63].T @ basis_k[:63]
  -> scores^T = Qt~^T Kt~ with 65-row operands: [Qt ; ||Q||], [C Kt ; ||K||-muk]
  (the matching muk*||Q|| term is constant along k -> softmax unchanged,
  centering keeps f32r products small).
  Scores are computed transposed ([k, q]); the softmax denominator comes from
  an appended ones-row in V (row 64) so PV needs no transposes.
  Softmax is max-free (scores bounded for this problem's scale); exp args are
  globally shifted by -20 which cancels in the normalization.
  The causal mask is applied by zeroing the upper triangle of diagonal exp
  tiles with gpsimd affine_select (on the otherwise idle Pool engine), and
  1/z is broadcast across partitions with gpsimd partition_broadcast.
  Both heads share one V projection / CK matmul (block-diagonal C) so that
  PE and copy work runs at 128 live partitions instead of 2x64.
"""

import sys

sys.path.insert(0, "/opt/trn_rl_repo")

import numpy as np

import concourse.bass as bass  # noqa: F401  (AP helpers)
import concourse.tile as tile
from concourse import bacc, mybir
from concourse.bass_utils import run_bass_kernel_spmd

F32 = mybir.dt.float32
F32R = mybir.dt.float32r
F16 = mybir.dt.float16
EXP = mybir.ActivationFunctionType.Exp
SQRT = mybir.ActivationFunctionType.Sqrt
CPY = mybir.ActivationFunctionType.Copy
IDN = mybir.ActivationFunctionType.Identity
ALU = mybir.AluOpType

B, L, D, H, HD = 2, 2048, 512, 8, 64
NC = 8
NL = 4                # l-slices of 512
NK = L // 128         # 16 k-tiles
HP = 2                # heads per core


def _build(causal: bool, repeat: int = 1):
    nc = bacc.Bacc("TRN2", target_bir_lowering=False, debug=False,
                   enable_asserts=True, num_devices=NC)

    xt = nc.dram_tensor("xt", [D, L], F32R, kind="ExternalInput")
    wqk = nc.dram_tensor("wqk", [128, HP, 4, 128], F32R, kind="ExternalInput")
    wv = nc.dram_tensor("wv", [128, 4, 128], F32R, kind="ExternalInput")
    wo = nc.dram_tensor("wo", [HD, HP, D], F32R, kind="ExternalInput")
    cmt = nc.dram_tensor("cmt", [128, 128], F32R, kind="ExternalInput")
    bq = nc.dram_tensor("bq", [HD, HP], F32, kind="ExternalInput")
    bk = nc.dram_tensor("bk", [HD, HP], F32, kind="ExternalInput")
    bv4 = nc.dram_tensor("bv4", [128, HP, 4 * HD], F32, kind="ExternalInput")
    muk2 = nc.dram_tensor("muk2", [2, 1], F32, kind="ExternalInput")
    on2 = nc.dram_tensor("on2", [128, 2], F32R, kind="ExternalInput")
    idm = nc.dram_tensor("idm", [128, 128], F32R, kind="ExternalInput")
    if not causal:
        maskf = nc.dram_tensor("maskf", [L, L], F32, kind="ExternalInput")
    yts = nc.dram_tensor("yts", [4, 128, L], F16, kind="ExternalOutput")

    from contextlib import ExitStack
    with tile.TileContext(nc) as tc, ExitStack() as ctx:
        ec = ctx.enter_context
        const = ec(tc.tile_pool(name="const", bufs=1))
        xtp = ec(tc.tile_pool(name="xtp", bufs=1))
        qtp = ec(tc.tile_pool(name="qtp", bufs=1))
        ktp = ec(tc.tile_pool(name="ktp", bufs=1))
        krawp = ec(tc.tile_pool(name="krawp", bufs=2))
        sqp = ec(tc.tile_pool(name="sqp", bufs=2))
        nrmp = ec(tc.tile_pool(name="nrmp", bufs=2))
        vttp = ec(tc.tile_pool(name="vttp", bufs=2))
        vtp = ec(tc.tile_pool(name="vtp", bufs=1))
        expp = ec(tc.tile_pool(name="expp", bufs=4))
        uscp = ec(tc.tile_pool(name="uscp", bufs=2))
        rzp = ec(tc.tile_pool(name="rzp", bufs=2))
        zbp = ec(tc.tile_pool(name="zbp", bufs=2))
        y16p = ec(tc.tile_pool(name="y16p", bufs=4))
        mldp = ec(tc.tile_pool(name="mldp", bufs=3)) if not causal else None
        pp = ec(tc.tile_pool(name="pp", bufs=4, space="PSUM"))
        sp = ec(tc.tile_pool(name="sp", bufs=2, space="PSUM"))
        up = ec(tc.tile_pool(name="up", bufs=2, space="PSUM"))

        # ---- constants (gpsimd queue; wqk on sync queue first, then xt) ----
        wqk_sb = const.tile([128, HP, 4, 128], F32R)
        wv_sb = const.tile([128, 4, 128], F32R)
        wo_sb = const.tile([HD, HP, D], F32R)
        cm_sb = const.tile([128, 128], F32R)
        bq_sb = const.tile([HD, HP], F32)
        bk_sb = const.tile([HD, HP], F32)
        bv_sb = const.tile([128, HP, 4 * HD], F32)
        muk_sb = const.tile([2, 1], F32)
        on2_sb = const.tile([128, 2], F32R)
        ident_sb = const.tile([128, 128], F32R)
        shift_sb = const.tile([128, 1], F32)
        nc.sync.dma_start(out=wqk_sb[:, :, :, :], in_=wqk[:, :, :, :])
        nc.gpsimd.dma_start(out=cm_sb[:, :], in_=cmt[:, :])
        nc.gpsimd.dma_start(out=on2_sb[:, :], in_=on2[:, :])
        nc.gpsimd.dma_start(out=wv_sb[:, :, :], in_=wv[:, :, :])
        nc.gpsimd.dma_start(out=bq_sb[:, :], in_=bq[:, :])
        nc.gpsimd.dma_start(out=bk_sb[:, :], in_=bk[:, :])
        nc.gpsimd.dma_start(out=muk_sb[:, :], in_=muk2[:, :])
        nc.gpsimd.dma_start(out=ident_sb[:, :], in_=idm[:, :])
        nc.gpsimd.dma_start(out=bv_sb[:, :, :], in_=bv4[:, :, :])
        nc.gpsimd.dma_start(out=wo_sb[:, :, :], in_=wo[:, :, :])
        nc.vector.memset(shift_sb[:, :], -20.0)

        for _rep in range(repeat):
            # xt SBUF: [128 part, 4 dc, 2048]; one DMA per 512-col l-slice.
            xts = xtp.tile([128, 4, L], F32R, tag="xts")
            for ls in range(NL):
                s = slice(512 * ls, 512 * (ls + 1))
                nc.sync.dma_start(
                    out=xts[:, :, s],
                    in_=xt[:, s].rearrange("(c p) m -> p c m", c=4))

            qt = [qtp.tile([HD + 1, L], F32R, tag=f"qt{h}", name=f"qt{h}")
                  for h in range(HP)]
            kt = [ktp.tile([HD + 1, L], F32R, tag=f"kt{h}", name=f"kt{h}")
                  for h in range(HP)]
            vt = [vtp.tile([128, NK, HD + 1], F32R, tag=f"vt{h}", name=f"vt{h}")
                  for h in range(HP)]
            usc = {}
            for h in range(HP):
                nc.vector.memset(vt[h][:, :, HD:HD + 1], 1.0)

            def proj_ls(ls):
                s = slice(512 * ls, 512 * (ls + 1))
                # fully per-head chains so head 0's attention never waits on
                # head 1's projection: per head h we produce qt_h, kt_h rows
                # and norms from its own qk matmuls / squares / CK matmul.
                for h in range(HP):
                    qk_ps = pp.tile([128, 512], F32, tag="pp")
                    for dc in range(4):
                        nc.tensor.matmul(qk_ps[:, :], wqk_sb[:, h, dc, :],
                                         xts[:, dc, s],
                                         start=(dc == 0), stop=(dc == 3))
                    nc.vector.tensor_scalar_add(qt[h][0:HD, s],
                                                qk_ps[0:HD, :],
                                                bq_sb[:, h:h + 1])
                    kraw = krawp.tile([HD, 512], F32R, tag="kraw")
                    nc.scalar.activation(kraw[:, :], qk_ps[HD:128, :],
                                         IDN, bias=bk_sb[:, h:h + 1])
                    # squares: rows 0:64 = q_h^2, 64:128 = k_h^2
                    sqh = sqp.tile([128, 512], F32R, tag="sqq")
                    nc.vector.tensor_mul(sqh[0:HD, :],
                                         qt[h][0:HD, s], qt[h][0:HD, s])
                    nc.vector.tensor_mul(sqh[HD:128, :], kraw[:, :], kraw[:, :])
                    ssq_ps = pp.tile([128, 512], F32, tag="pp")
                    nc.tensor.matmul(ssq_ps[0:2, :], on2_sb[:, :], sqh[:, :],
                                     start=True, stop=True)
                    lnt = nrmp.tile([2, 512], F32, tag="lnt")
                    nc.scalar.activation(lnt[:, :], ssq_ps[0:2, :], LN)
                    nrm2 = nrmp.tile([2, 512], F32R, tag="nrm")
                    nc.scalar.activation(nrm2[:, :], lnt[:, :], EXP, scale=0.5)
                    # row 0 = ||q||: straight into qt; row 1 = ||k|| sits at
                    # partition base 1 (illegal for ALU reads) - replicate to
                    # base 0 with a tiny PE selector, then subtract muk.
                    selk_ps = pp.tile([128, 512], F32, tag="pp")
                    nc.tensor.matmul(selk_ps[0:HD, :], sel1_sb[:, :],
                                     nrm2[0:2, :], start=True, stop=True)
                    nc.vector.tensor_copy(qt[h][HD:HD + 1, s], nrm2[0:1, :])
                    nc.vector.tensor_scalar_sub(
                        kt[h][HD:HD + 1, s], selk_ps[0:1, :],
                        muk_sb[32 * h:32 * h + 1, 0:1])
                    ck_ps = pp.tile([128, 512], F32, tag="pp")
                    nc.tensor.matmul(ck_ps[0:HD, :], cm_sb[0:HD, 0:HD],
                                     kraw[:, :], start=True, stop=True)
                    nc.scalar.activation(kt[h][0:HD, s], ck_ps[0:HD, :], CPY)
                # V projection for both heads (V_A rows 0:64, V_B 64:128)
                vt_ps = pp.tile([128, 512], F32, tag="pp")
                for dc in range(4):
                    nc.tensor.matmul(vt_ps[:, :], wv_sb[:, dc, :],
                                     xts[:, dc, s], start=(dc == 0), stop=(dc == 3))
                vtt = vttp.tile([128, 512], F32R, tag="vtt")
                nc.scalar.activation(vtt[:, :], vt_ps[:, :], CPY)
                v4_ps = pp.tile([128, 512], F32, tag="pp")
                for r in range(4):
                    nc.tensor.transpose(v4_ps[:, 128 * r:128 * (r + 1)].bitcast(F32R),
                                        vtt[:, 128 * r:128 * (r + 1)],
                                        ident_sb[:, :])
                v4v = v4_ps[:, :].rearrange("p (r d) -> p r d", r=4)
                for h in range(HP):
                    nc.vector.tensor_add(
                        vt[h][:, 4 * ls:4 * (ls + 1), 0:HD],
                        v4v[:, :, HD * h:HD * (h + 1)],
                        bv_sb[:, h, :].rearrange("p (r d) -> p r d", r=4))

            def attention(h, n):
                qs = slice(512 * n, 512 * (n + 1))
                kmax = 4 * (n + 1) if causal else NK
                u_ps = up.tile([HD + 1, 512], F32, tag="up")
                if causal:
                    kis = list(range(4 * n, kmax)) + list(range(0, 4 * n))
                else:
                    kis = list(range(kmax))
                first_ki, last_ki = kis[0], kis[-1]
                for ki in kis:
                    lo = 0
                    if causal and 4 * n <= ki <= 4 * n + 3:
                        lo = 128 * (ki - 4 * n)
                    w = slice(lo, 512)
                    st_ps = sp.tile([128, 512], F32, tag="sp")
                    nc.tensor.matmul(st_ps[:, w],
                                     kt[h][:, 128 * ki:128 * (ki + 1)],
                                     qt[h][:, qs][:, w], start=True, stop=True)
                    if not causal:
                        mld = mldp.tile([128, 512], F32)
                        nc.sync.dma_start(out=mld[:, :],
                                          in_=maskf[128 * ki:128 * (ki + 1), qs])
                        nc.vector.tensor_add(st_ps[:, :], st_ps[:, :], mld[:, :])
                    ex = expp.tile([128, 512], F32R)
                    # global shift keeps exp in fp32 range; cancels in the
                    # normalization.
                    nc.scalar.activation(ex[:, w], st_ps[:, w], EXP,
                                         scale=0.125, bias=shift_sb[:, 0:1])
                    if causal and 4 * n <= ki <= 4 * n + 3:
                        # zero the upper triangle of the diagonal block
                        # (q_local - k_part < 0) on the idle gpsimd engine
                        nc.gpsimd.affine_select(
                            ex[:, lo:lo + 128].bitcast(F32),
                            ex[:, lo:lo + 128].bitcast(F32),
                            [[1, 128]], ALU.is_ge, 0.0,
                            base=0, channel_multiplier=-1)
                    nc.tensor.matmul(u_ps[:, w], vt[h][:, ki, :], ex[:, w],
                                     start=(ki == first_ki),
                                     stop=(ki == last_ki))
                rz = rzp.tile([1, 512], F32R, tag="rz")
                with nc.allow_low_precision(reason="f32r rounding of softmax denom"):
                    nc.vector.reciprocal(rz[:, :], u_ps[HD:HD + 1, :])
                zb = zbp.tile([HD, 512], F32, tag="zb")
                nc.gpsimd.partition_broadcast(zb[:, :], rz[:, :].bitcast(F32))
                u = uscp.tile([HD, 512], F32R, tag=f"usc{h}", name=f"usc{h}")
                nc.vector.tensor_mul(u[:, :], u_ps[0:HD, :], zb[:, :])
                usc[(h, n)] = u

            def outproj(n):
                qs = slice(512 * n, 512 * (n + 1))
                for dt_ in range(4):
                    y_ps = pp.tile([128, 512], F32, tag="pp")
                    for h in range(HP):
                        nc.tensor.matmul(y_ps[:, :],
                                         wo_sb[:, h, 128 * dt_:128 * (dt_ + 1)],
                                         usc[(h, n)][:, :],
                                         start=(h == 0), stop=(h == HP - 1))
                    y16 = y16p.tile([128, 512], F16, tag="y16")
                    if dt_ % 2 == 0:
                        nc.scalar.activation(y16[:, :], y_ps[:, :], CPY)
                    else:
                        nc.vector.tensor_copy(y16[:, :], y_ps[:, :])
                    nc.sync.dma_start(out=yts[dt_, :, qs], in_=y16[:, :])

            for n in range(NL):
                proj_ls(n)
                for h in range(HP):
                    attention(h, n)
                outproj(n)
    nc.compile()
    return nc


_CACHE = {}


def _get(causal: bool, repeat: int = 1):
    key = (causal, repeat)
    if key not in _CACHE:
        _CACHE[key] = _build(causal, repeat)
    return _CACHE[key]


def _make_w(coef):
    iu = np.triu_indices(D, k=1)
    a = np.zeros((D, D), np.float32)
    a[iu] = coef
    return a - a.T + np.eye(D, dtype=np.float32)


def _prep(x, mask, coef_q, coef_k, coef_v, coef_o,
          bias_q, bias_k, bias_v, bias_o, basis_q, basis_k):
    x = np.asarray(x, np.float32)
    mask = np.asarray(mask, np.float32)
    wq, wk, wv, wo = (_make_w(np.asarray(c, np.float32))
                      for c in (coef_q, coef_k, coef_v, coef_o))
    basis_q = np.asarray(basis_q, np.float32)
    basis_k = np.asarray(basis_k, np.float32)
    cmt = np.ascontiguousarray(
        basis_k[:HD - 1, :].T @ basis_q[:HD - 1, :]).astype(np.float32)

    # causal fast path: mask[q, k] == 0 for k <= q else -1e9
    ii = np.arange(L)
    causal_ref = np.where(ii[None, :] <= ii[:, None], 0.0, -1e9).astype(np.float32)
    causal = bool(np.array_equal(mask, causal_ref))

    on2 = np.zeros((128, 2), np.float32)
    on2[0:HD, 0] = 1.0
    on2[HD:128, 1] = 1.0
    cm2 = np.zeros((128, 128), np.float32)
    cm2[0:HD, 0:HD] = cmt
    cm2[HD:128, HD:128] = cmt
    shared = {
        "on2": on2,
        "cmt": cm2,
        "idm": np.eye(128, dtype=np.float32),
    }
    if not causal:
        shared["maskf"] = np.ascontiguousarray(8.0 * mask.T)

    bias_q = np.asarray(bias_q, np.float32)
    bias_k = np.asarray(bias_k, np.float32)
    bias_v = np.asarray(bias_v, np.float32)

    in_maps = []
    for c in range(NC):
        b, hp = c // 4, c % 4
        heads = (2 * hp, 2 * hp + 1)
        m = dict(shared)
        m["xt"] = np.ascontiguousarray(x[b].T)
        wqk_l, wo_l, bq_l, bk_l, bv_l, muk_l, wv_l = [], [], [], [], [], [], []
        for h in heads:
            hs = slice(HD * h, HD * (h + 1))
            wqkt = np.concatenate([wq[hs, :].T, wk[hs, :].T], axis=1)  # [512,128]
            wqk_l.append(wqkt.reshape(4, 128, 128).transpose(1, 0, 2))
            wv_l.append(wv[hs, :].T)                                    # [512,64]
            wo_l.append(np.ascontiguousarray(wo[:, hs].T))              # [64,512]
            bq_l.append(bias_q[hs])
            bk_l.append(bias_k[hs])
            bv_l.append(np.broadcast_to(
                np.tile(bias_v[hs], 4)[None, :], (128, 4 * HD)))
            muk_l.append(np.linalg.norm(wk[hs, :]))
        m["wqk"] = np.ascontiguousarray(np.stack(wqk_l, axis=1))
        # shared V projection: outputs [V_A(64) | V_B(64)]
        wvab = np.concatenate(wv_l, axis=1)                             # [512,128]
        m["wv"] = np.ascontiguousarray(
            wvab.reshape(4, 128, 128).transpose(1, 0, 2))
        m["wo"] = np.ascontiguousarray(np.stack(wo_l, axis=1))
        m["bq"] = np.ascontiguousarray(np.stack(bq_l, axis=1))
        m["bk"] = np.ascontiguousarray(np.stack(bk_l, axis=1))
        m["bv4"] = np.ascontiguousarray(np.stack(bv_l, axis=1))
        m["muk2"] = np.array([[muk_l[0]], [muk_l[1]]], np.float32)
        in_maps.append(m)
    return causal, in_maps


def kernel(_trace=False, **inputs):
    causal, in_maps = _prep(**inputs)
    nc = _get(causal)
    res = run_bass_kernel_spmd(nc, in_maps, list(range(NC)), trace=_trace)
    bias_o = np.asarray(inputs["bias_o"], np.float32)
    y = np.zeros((B, D, L), np.float32)
    for c in range(NC):
        yp = np.asarray(res.results[c]["yts"], np.float16).astype(np.float32)
        y[c // 4] += yp.reshape(D, L)
    out = y.transpose(0, 2, 1) + bias_o[None, None, :]
    if _trace:
        kernel._last = res
    return np.ascontiguousarray(out)


def bench(inputs, repeats=(1, 5), iters=5):
    """Per-iteration HW-ish time via repeat-differencing (no NTFF here)."""
    import time as _t
    causal, in_maps = _prep(**inputs)
    walls = {}
    for rep in repeats:
        nc = _get(causal, rep)
        run_bass_kernel_spmd(nc, in_maps, list(range(NC)))  # warm (compile+cache)
        best = float("inf")
        for _ in range(iters):
            t0 = _t.perf_counter()
            run_bass_kernel_spmd(nc, in_maps, list(range(NC)))
            best = min(best, _t.perf_counter() - t0)
        walls[rep] = best
    r0, r1 = min(repeats), max(repeats)
    per_iter_ns = (walls[r1] - walls[r0]) / (r1 - r0) * 1e9
    return per_iter_ns, walls


# revision 24
# speedup vs baseline: 1.0004x; 1.0004x over previous
"""EquivariantAttention Trainium2 kernel.

B=2, L=2048, D=512, H=8, HD=64 over 8 NeuronCores.
Each core owns ONE batch and TWO heads (core c: batch c//4, heads
{2*(c%4), 2*(c%4)+1}); it computes those heads' attention plus their
partial contribution to the output projection y_c = sum_h Wo_h^T usc_h
over the full sequence.  The host gather sums the 4 per-batch partials
(no on-device collective at all).

Math notes:
  Qi . Ki = ||Q_l||*||K_m|| + Q_l^T C K_m,  C = basis_q[:63].T @ basis_k[:63]
  -> scores^T = Qt~^T Kt~ with 65-row operands: [Qt ; ||Q||], [C Kt ; ||K||-muk]
  (the matching muk*||Q|| term is constant along k -> softmax unchanged,
  centering keeps f32r products small).
  Scores are computed transposed ([k, q]); the softmax denominator comes from
  an appended ones-row in V (row 64) so PV needs no transposes.
  Softmax is max-free (scores bounded for this problem's scale); exp args are
  globally shifted by -20 which cancels in the normalization.
  The causal mask is applied by zeroing the upper triangle of diagonal exp
  tiles with gpsimd affine_select (on the otherwise idle Pool engine), and
  1/z is broadcast across partitions with gpsimd partition_broadcast.
  Both heads share one V projection / CK matmul (block-diagonal C) so that
  PE and copy work runs at 128 live partitions instead of 2x64.
"""

import sys

sys.path.insert(0, "/opt/trn_rl_repo")

import numpy as np

import concourse.bass as bass  # noqa: F401  (AP helpers)
import concourse.tile as tile
from concourse import bacc, mybir
from concourse.bass_utils import run_bass_kernel_spmd

F32 = mybir.dt.float32
F32R = mybir.dt.float32r
F16 = mybir.dt.float16
EXP = mybir.ActivationFunctionType.Exp
SQRT = mybir.ActivationFunctionType.Sqrt
CPY = mybir.ActivationFunctionType.Copy
IDN = mybir.ActivationFunctionType.Identity
ALU = mybir.AluOpType

B, L, D, H, HD = 2, 2048, 512, 8, 64
NC = 8
NL = 4                # l-slices of 512
NK = L // 128         # 16 k-tiles
HP = 2                # heads per core


def _build(causal: bool, repeat: int = 1):
    nc = bacc.Bacc("TRN2", target_bir_lowering=False, debug=False,
                   enable_asserts=True, num_devices=NC)

    xt = nc.dram_tensor("xt", [D, L], F32R, kind="ExternalInput")
    wqk = nc.dram_tensor("wqk", [128, HP, 4, 128], F32R, kind="ExternalInput")
    wv = nc.dram_tensor("wv", [128, 4, 128], F32R, kind="ExternalInput")
    wo = nc.dram_tensor("wo", [HD, HP, D], F32R, kind="ExternalInput")
    cmt = nc.dram_tensor("cmt", [128, 128], F32R, kind="ExternalInput")
    bq = nc.dram_tensor("bq", [HD, HP], F32, kind="ExternalInput")
    bk = nc.dram_tensor("bk", [HD, HP], F32, kind="ExternalInput")
    bv4 = nc.dram_tensor("bv4", [128, HP, 4 * HD], F32, kind="ExternalInput")
    muk2 = nc.dram_tensor("muk2", [2, 1], F32, kind="ExternalInput")
    on2 = nc.dram_tensor("on2", [128, 2], F32R, kind="ExternalInput")
    idm = nc.dram_tensor("idm", [128, 128], F32R, kind="ExternalInput")
    if not causal:
        maskf = nc.dram_tensor("maskf", [L, L], F32, kind="ExternalInput")
    yts = nc.dram_tensor("yts", [4, 128, L], F16, kind="ExternalOutput")

    from contextlib import ExitStack
    with tile.TileContext(nc) as tc, ExitStack() as ctx:
        ec = ctx.enter_context
        const = ec(tc.tile_pool(name="const", bufs=1))
        xtp = ec(tc.tile_pool(name="xtp", bufs=1))
        qtp = ec(tc.tile_pool(name="qtp", bufs=1))
        ktp = ec(tc.tile_pool(name="ktp", bufs=1))
        krawp = ec(tc.tile_pool(name="krawp", bufs=2))
        sqp = ec(tc.tile_pool(name="sqp", bufs=2))
        nrmp = ec(tc.tile_pool(name="nrmp", bufs=2))
        vttp = ec(tc.tile_pool(name="vttp", bufs=2))
        vtp = ec(tc.tile_pool(name="vtp", bufs=1))
        expp = ec(tc.tile_pool(name="expp", bufs=4))
        uscp = ec(tc.tile_pool(name="uscp", bufs=2))
        rzp = ec(tc.tile_pool(name="rzp", bufs=2))
        zbp = ec(tc.tile_pool(name="zbp", bufs=2))
        y16p = ec(tc.tile_pool(name="y16p", bufs=4))
        mldp = ec(tc.tile_pool(name="mldp", bufs=3)) if not causal else None
        pp = ec(tc.tile_pool(name="pp", bufs=4, space="PSUM"))
        sp = ec(tc.tile_pool(name="sp", bufs=2, space="PSUM"))
        up = ec(tc.tile_pool(name="up", bufs=2, space="PSUM"))

        # ---- constants (gpsimd queue; wqk on sync queue first, then xt) ----
        wqk_sb = const.tile([128, HP, 4, 128], F32R)
        wv_sb = const.tile([128, 4, 128], F32R)
        wo_sb = const.tile([HD, HP, D], F32R)
        cm_sb = const.tile([128, 128], F32R)
        bq_sb = const.tile([HD, HP], F32)
        bk_sb = const.tile([HD, HP], F32)
        bv_sb = const.tile([128, HP, 4 * HD], F32)
        muk_sb = const.tile([2, 1], F32)
        on2_sb = const.tile([128, 2], F32R)
        ident_sb = const.tile([128, 128], F32R)
        shift_sb = const.tile([128, 1], F32)
        nc.sync.dma_start(out=wqk_sb[:, :, :, :], in_=wqk[:, :, :, :])
        nc.gpsimd.dma_start(out=cm_sb[:, :], in_=cmt[:, :])
        nc.gpsimd.dma_start(out=on2_sb[:, :], in_=on2[:, :])
        nc.gpsimd.dma_start(out=wv_sb[:, :, :], in_=wv[:, :, :])
        nc.gpsimd.dma_start(out=bq_sb[:, :], in_=bq[:, :])
        nc.gpsimd.dma_start(out=bk_sb[:, :], in_=bk[:, :])
        nc.gpsimd.dma_start(out=muk_sb[:, :], in_=muk2[:, :])
        nc.gpsimd.dma_start(out=ident_sb[:, :], in_=idm[:, :])
        nc.gpsimd.dma_start(out=bv_sb[:, :, :], in_=bv4[:, :, :])
        nc.gpsimd.dma_start(out=wo_sb[:, :, :], in_=wo[:, :, :])
        nc.vector.memset(shift_sb[:, :], -20.0)

        for _rep in range(repeat):
            # xt SBUF: [128 part, 4 dc, 2048]; one DMA per 512-col l-slice.
            xts = xtp.tile([128, 4, L], F32R, tag="xts")
            for ls in range(NL):
                s = slice(512 * ls, 512 * (ls + 1))
                nc.sync.dma_start(
                    out=xts[:, :, s],
                    in_=xt[:, s].rearrange("(c p) m -> p c m", c=4))

            qt = [qtp.tile([HD + 1, L], F32R, tag=f"qt{h}", name=f"qt{h}")
                  for h in range(HP)]
            kt = [ktp.tile([HD + 1, L], F32R, tag=f"kt{h}", name=f"kt{h}")
                  for h in range(HP)]
            vt = [vtp.tile([128, NK, HD + 1], F32R, tag=f"vt{h}", name=f"vt{h}")
                  for h in range(HP)]
            usc = {}
            for h in range(HP):
                nc.vector.memset(vt[h][:, :, HD:HD + 1], 1.0)

            def proj_ls(ls):
                s = slice(512 * ls, 512 * (ls + 1))
                # fully per-head chains so head 0's attention never waits on
                # head 1's projection: per head h we produce qt_h, kt_h rows
                # and norms from its own qk matmuls / squares / CK matmul.
                for h in range(HP):
                    qk_ps = pp.tile([128, 512], F32, tag="pp")
                    for dc in range(4):
                        nc.tensor.matmul(qk_ps[:, :], wqk_sb[:, h, dc, :],
                                         xts[:, dc, s],
                                         start=(dc == 0), stop=(dc == 3))
                    nc.vector.tensor_scalar_add(qt[h][0:HD, s],
                                                qk_ps[0:HD, :],
                                                bq_sb[:, h:h + 1])
                    kraw = krawp.tile([HD, 512], F32R, tag="kraw")
                    nc.scalar.activation(kraw[:, :], qk_ps[HD:128, :],
                                         IDN, bias=bk_sb[:, h:h + 1])
                    # squares: rows 0:64 = q_h^2, 64:128 = k_h^2
                    sqh = sqp.tile([128, 512], F32R, tag="sqq")
                    nc.vector.tensor_mul(sqh[0:HD, :],
                                         qt[h][0:HD, s], qt[h][0:HD, s])
                    nc.vector.tensor_mul(sqh[HD:128, :], kraw[:, :], kraw[:, :])
                    ssq_ps = pp.tile([128, 512], F32, tag="pp")
                    nc.tensor.matmul(ssq_ps[0:2, :], on2_sb[:, :], sqh[:, :],
                                     start=True, stop=True)
                    lnt = nrmp.tile([2, 512], F32, tag="lnt")
                    nc.scalar.activation(lnt[:, :], ssq_ps[0:2, :], LN)
                    nrm2 = nrmp.tile([2, 512], F32R, tag="nrm")
                    nc.scalar.activation(nrm2[:, :], lnt[:, :], EXP, scale=0.5)
                    # row 0 = ||q||: straight into qt; row 1 = ||k|| sits at
                    # partition base 1 (illegal for ALU reads) - replicate to
                    # base 0 with a tiny PE selector, then subtract muk.
                    selk_ps = pp.tile([128, 512], F32, tag="pp")
                    nc.tensor.matmul(selk_ps[0:HD, :], sel1_sb[:, :],
                                     nrm2[0:2, :], start=True, stop=True)
                    nc.vector.tensor_copy(qt[h][HD:HD + 1, s], nrm2[0:1, :])
                    nc.vector.tensor_scalar_sub(
                        kt[h][HD:HD + 1, s], selk_ps[0:1, :],
                        muk_sb[32 * h:32 * h + 1, 0:1])
                    ck_ps = pp.tile([128, 512], F32, tag="pp")
                    nc.tensor.matmul(ck_ps[0:HD, :], cm_sb[0:HD, 0:HD],
                                     kraw[:, :], start=True, stop=True)
                    nc.scalar.activation(kt[h][0:HD, s], ck_ps[0:HD, :], CPY)
                # V projection for both heads (V_A rows 0:64, V_B 64:128)
                vt_ps = pp.tile([128, 512], F32, tag="pp")
                for dc in range(4):
                    nc.tensor.matmul(vt_ps[:, :], wv_sb[:, dc, :],
                                     xts[:, dc, s], start=(dc == 0), stop=(dc == 3))
                vtt = vttp.tile([128, 512], F32R, tag="vtt")
                nc.scalar.activation(vtt[:, :], vt_ps[:, :], CPY)
                v4_ps = pp.tile([128, 512], F32, tag="pp")
                for r in range(4):
                    nc.tensor.transpose(v4_ps[:, 128 * r:128 * (r + 1)].bitcast(F32R),
                                        vtt[:, 128 * r:128 * (r + 1)],
                                        ident_sb[:, :])
                v4v = v4_ps[:, :].rearrange("p (r d) -> p r d", r=4)
                for h in range(HP):
                    nc.vector.tensor_add(
                        vt[h][:, 4 * ls:4 * (ls + 1), 0:HD],
                        v4v[:, :, HD * h:HD * (h + 1)],
                        bv_sb[:, h, :].rearrange("p (r d) -> p r d", r=4))

            def attention(h, n):
                qs = slice(512 * n, 512 * (n + 1))
                kmax = 4 * (n + 1) if causal else NK
                u_ps = up.tile([HD + 1, 512], F32, tag="up")
                if causal:
                    kis = list(range(4 * n, kmax)) + list(range(0, 4 * n))
                else:
                    kis = list(range(kmax))
                first_ki, last_ki = kis[0], kis[-1]
                for ki in kis:
                    lo = 0
                    if causal and 4 * n <= ki <= 4 * n + 3:
                        lo = 128 * (ki - 4 * n)
                    w = slice(lo, 512)
                    st_ps = sp.tile([128, 512], F32, tag="sp")
                    nc.tensor.matmul(st_ps[:, w],
                                     kt[h][:, 128 * ki:128 * (ki + 1)],
                                     qt[h][:, qs][:, w], start=True, stop=True)
                    if not causal:
                        mld = mldp.tile([128, 512], F32)
                        nc.sync.dma_start(out=mld[:, :],
                                          in_=maskf[128 * ki:128 * (ki + 1), qs])
                        nc.vector.tensor_add(st_ps[:, :], st_ps[:, :], mld[:, :])
                    ex = expp.tile([128, 512], F32R)
                    # global shift keeps exp in fp32 range; cancels in the
                    # normalization.
                    nc.scalar.activation(ex[:, w], st_ps[:, w], EXP,
                                         scale=0.125, bias=shift_sb[:, 0:1])
                    if causal and 4 * n <= ki <= 4 * n + 3:
                        # zero the upper triangle of the diagonal block
                        # (q_local - k_part < 0) on the idle gpsimd engine
                        nc.gpsimd.affine_select(
                            ex[:, lo:lo + 128].bitcast(F32),
                            ex[:, lo:lo + 128].bitcast(F32),
                            [[1, 128]], ALU.is_ge, 0.0,
                            base=0, channel_multiplier=-1)
                    nc.tensor.matmul(u_ps[:, w], vt[h][:, ki, :], ex[:, w],
                                     start=(ki == first_ki),
                                     stop=(ki == last_ki))
                rz = rzp.tile([1, 512], F32R, tag="rz")
                with nc.allow_low_precision(reason="f32r rounding of softmax denom"):
                    nc.vector.reciprocal(rz[:, :], u_ps[HD:HD + 1, :])
                zb = zbp.tile([HD, 512], F32, tag="zb")
                nc.gpsimd.partition_broadcast(zb[:, :], rz[:, :].bitcast(F32))
                u = uscp.tile([HD, 512], F32R, tag=f"usc{h}", name=f"usc{h}")
                nc.vector.tensor_mul(u[:, :], u_ps[0:HD, :], zb[:, :])
                usc[(h, n)] = u

            def outproj(n):
                qs = slice(512 * n, 512 * (n + 1))
                for dt_ in range(4):
                    y_ps = pp.tile([128, 512], F32, tag="pp")
                    for h in range(HP):
                        nc.tensor.matmul(y_ps[:, :],
                                         wo_sb[:, h, 128 * dt_:128 * (dt_ + 1)],
                                         usc[(h, n)][:, :],
                                         start=(h == 0), stop=(h == HP - 1))
                    y16 = y16p.tile([128, 512], F16, tag="y16")
                    if dt_ % 2 == 0:
                        nc.scalar.activation(y16[:, :], y_ps[:, :], CPY)
                    else:
                        nc.vector.tensor_copy(y16[:, :], y_ps[:, :])
                    nc.sync.dma_start(out=yts[dt_, :, qs], in_=y16[:, :])

            for n in range(NL):
                proj_ls(n)
                for h in range(HP):
                    attention(h, n)
                outproj(n)
    nc.compile()
    return nc


_CACHE = {}


def _get(causal: bool, repeat: int = 1):
    key = (causal, repeat)
    if key not in _CACHE:
        _CACHE[key] = _build(causal, repeat)
    return _CACHE[key]


def _make_w(coef):
    iu = np.triu_indices(D, k=1)
    a = np.zeros((D, D), np.float32)
    a[iu] = coef
    return a - a.T + np.eye(D, dtype=np.float32)


def _prep(x, mask, coef_q, coef_k, coef_v, coef_o,
          bias_q, bias_k, bias_v, bias_o, basis_q, basis_k):
    x = np.asarray(x, np.float32)
    mask = np.asarray(mask, np.float32)
    wq, wk, wv, wo = (_make_w(np.asarray(c, np.float32))
                      for c in (coef_q, coef_k, coef_v, coef_o))
    basis_q = np.asarray(basis_q, np.float32)
    basis_k = np.asarray(basis_k, np.float32)
    cmt = np.ascontiguousarray(
        basis_k[:HD - 1, :].T @ basis_q[:HD - 1, :]).astype(np.float32)

    # causal fast path: mask[q, k] == 0 for k <= q else -1e9
    ii = np.arange(L)
    causal_ref = np.where(ii[None, :] <= ii[:, None], 0.0, -1e9).astype(np.float32)
    causal = bool(np.array_equal(mask, causal_ref))

    on2 = np.zeros((128, 2), np.float32)
    on2[0:HD, 0] = 1.0
    on2[HD:128, 1] = 1.0
    cm2 = np.zeros((128, 128), np.float32)
    cm2[0:HD, 0:HD] = cmt
    cm2[HD:128, HD:128] = cmt
    shared = {
        "on2": on2,
        "cmt": cm2,
        "idm": np.eye(128, dtype=np.float32),
    }
    if not causal:
        shared["maskf"] = np.ascontiguousarray(8.0 * mask.T)

    bias_q = np.asarray(bias_q, np.float32)
    bias_k = np.asarray(bias_k, np.float32)
    bias_v = np.asarray(bias_v, np.float32)

    in_maps = []
    for c in range(NC):
        b, hp = c // 4, c % 4
        heads = (2 * hp, 2 * hp + 1)
        m = dict(shared)
        m["xt"] = np.ascontiguousarray(x[b].T)
        wqk_l, wo_l, bq_l, bk_l, bv_l, muk_l, wv_l = [], [], [], [], [], [], []
        for h in heads:
            hs = slice(HD * h, HD * (h + 1))
            wqkt = np.concatenate([wq[hs, :].T, wk[hs, :].T], axis=1)  # [512,128]
            wqk_l.append(wqkt.reshape(4, 128, 128).transpose(1, 0, 2))
            wv_l.append(wv[hs, :].T)                                    # [512,64]
            wo_l.append(np.ascontiguousarray(wo[:, hs].T))              # [64,512]
            bq_l.append(bias_q[hs])
            bk_l.append(bias_k[hs])
            bv_l.append(np.broadcast_to(
                np.tile(bias_v[hs], 4)[None, :], (128, 4 * HD)))
            muk_l.append(np.linalg.norm(wk[hs, :]))
        m["wqk"] = np.ascontiguousarray(np.stack(wqk_l, axis=1))
        # shared V projection: outputs [V_A(64) | V_B(64)]
        wvab = np.concatenate(wv_l, axis=1)                             # [512,128]
        m["wv"] = np.ascontiguousarray(
            wvab.reshape(4, 128, 128).transpose(1, 0, 2))
        m["wo"] = np.ascontiguousarray(np.stack(wo_l, axis=1))
        m["bq"] = np.ascontiguousarray(np.stack(bq_l, axis=1))
        m["bk"] = np.ascontiguousarray(np.stack(bk_l, axis=1))
        m["bv4"] = np.ascontiguousarray(np.stack(bv_l, axis=1))
        m["muk2"] = np.array([[muk_l[0]], [muk_l[1]]], np.float32)
        in_maps.append(m)
    return causal, in_maps


def kernel(_trace=False, **inputs):
    causal, in_maps = _prep(**inputs)
    nc = _get(causal)
    res = run_bass_kernel_spmd(nc, in_maps, list(range(NC)), trace=_trace)
    bias_o = np.asarray(inputs["bias_o"], np.float32)
    y = np.zeros((B, D, L), np.float32)
    for c in range(NC):
        yp = np.asarray(res.results[c]["yts"], np.float16).astype(np.float32)
        y[c // 4] += yp.reshape(D, L)
    out = y.transpose(0, 2, 1) + bias_o[None, None, :]
    if _trace:
        kernel._last = res
    return np.ascontiguousarray(out)


def bench(inputs, repeats=(1, 5), iters=5):
    """Per-iteration HW-ish time via repeat-differencing (no NTFF here)."""
    import time as _t
    causal, in_maps = _prep(**inputs)
    walls = {}
    for rep in repeats:
        nc = _get(causal, rep)
        run_bass_kernel_spmd(nc, in_maps, list(range(NC)))  # warm (compile+cache)
        best = float("inf")
        for _ in range(iters):
            t0 = _t.perf_counter()
            run_bass_kernel_spmd(nc, in_maps, list(range(NC)))
            best = min(best, _t.perf_counter() - t0)
        walls[rep] = best
    r0, r1 = min(repeats), max(repeats)
    per_iter_ns = (walls[r1] - walls[r0]) / (r1 - r0) * 1e9
    return per_iter_ns, walls
